# revision 17
# baseline (speedup 1.0000x reference)
"""AlignBlock Trainium2 kernel — 8-core SPMD, no collectives.

Sharding: 8 cores = 2 batch x 4 time-chunks of 100 steps, fully independent
(halo-included input slices).

Device algorithm per core ("shifted K-conv variants"):
  The 5x3 conv over (t, d) of the QK^T scores is folded EXACTLY into the
  score matmul by pre-convolving the K projection with the 3 d-taps for each
  of the 5 time taps i:

      KG_i[k, y] = sum_j' wc[h,i,j'] * Kh[k, y + j' - 1]          (k = (h,f))
      Ck[x, j]   = sum_i sum_k Q[k, x+i-4] * KG_i[k, j+i-4]

  The Q-side time shifts are free SBUF column offsets of one shared Q buffer
  (104 cols); the K-side shifts are baked into each variant's column layout.
  This ships 5x fp8 K-variants + 1x bf16 Q (3.4 MB) instead of rank-5 SVD
  factors of both sides (4.8 MB). KG is scaled by 64 (inverse folded into
  bf16 Q) to clear fp8e4m3's subnormal floor.

  The additive softmax mask (band + exact d-edge leak corrections + conv
  bias) is folded into the same PSUM accumulation as an identity-weighted
  bf16 matmul, so softmax is just exp() on ACT straight out of PSUM (logits
  bounded, no max pass). Attention weights are transposed on the PE and
  applied to raw bf16 x_ref windows in two stationary-weight rounds over 6
  output column chunks; 1/rowsum rides the PSUM->SBUF output copies (bf16
  out, halving the store).

  DMA notes: all bulk transfers use >=2.6KB-per-partition descriptors
  (smaller descriptors measurably drop per-engine DMA throughput); inputs
  are split across both HWDGE rings roughly proportional to when the
  tensor engine consumes them; the two output pieces ride opposite rings.
"""

import numpy as np
import ml_dtypes

B, C, H, T, F, DELAY = 2, 16, 16, 400, 161, 100
TL = 100            # output timesteps per core
QT = TL + 4         # mic-side cols (causal conv halo)
KT = TL + 103       # ref-side cols (window + conv halos)
NV = 5              # conv time taps = K variants
NCH = 21            # 128-row chunks per variant (H*F = 2576 rows)
TOTCH = NV * NCH    # 105
KSCALE = 64.0       # fp8 pre-scale on KG, inverse folded into Q
NEG = -60.0         # out-of-band additive mask
VB = [0, 432, 864, 1296, 1728, 2160, 2576]   # value/output column chunks
NPO = 5             # PSUM banks for value chunks (chunk 5 reuses bank 0)
GA = [(0, 13), (13, 39), (65, 89)]     # KG chunk groups on the sync ring
GB = [(39, 65), (89, 105)]             # KG chunk groups on the scalar ring
OSPL = VB[3]        # output piece split (chunks 0-2 / 3-5)

BF16 = ml_dtypes.bfloat16
FP8 = ml_dtypes.float8_e4m3

_CACHE = {}


def _build_raw():
    if "ncr" in _CACHE:
        return _CACHE["ncr"]
    import concourse.bass as bass
    from concourse import bacc, mybir

    dt = mybir.dt
    nc = bacc.Bacc("TRN2", target_bir_lowering=False, debug=False, num_devices=8)

    cm_d = nc.dram_tensor("cm", [128, 331], dt.bfloat16, kind="ExternalInput").ap()
    q_d = nc.dram_tensor("qf", [128, NCH, QT], dt.bfloat16, kind="ExternalInput").ap()
    kg_d = nc.dram_tensor("kg", [128, TOTCH, KT], dt.float8e4, kind="ExternalInput").ap()
    xr_d = nc.dram_tensor("xr", [KT, C * F], dt.bfloat16, kind="ExternalInput").ap()
    out_d = nc.dram_tensor("out", [TL, C * F], dt.bfloat16, kind="ExternalOutput").ap()

    # static SBUF
    cmb = nc.alloc_sbuf_tensor("cmb", [128, 331], dt.bfloat16).ap()
    qb = nc.alloc_sbuf_tensor("qb", [128, NCH, QT], dt.bfloat16).ap()
    kgb = nc.alloc_sbuf_tensor("kgb", [128, TOTCH, KT], dt.float8e4).ap()
    xr01 = nc.alloc_sbuf_tensor("xr01", [128, 2, C * F], dt.bfloat16).ap()
    eb = nc.alloc_sbuf_tensor("eb", [TL, KT], dt.bfloat16).ap()
    ssum = nc.alloc_sbuf_tensor("ssum", [TL, 1], dt.float32).ap()
    rinv = nc.alloc_sbuf_tensor("rinv", [TL, 1], dt.float32).ap()
    a0 = nc.alloc_sbuf_tensor("a0", [128, TL], dt.bfloat16).ap()
    a1 = nc.alloc_sbuf_tensor("a1", [KT - 128, TL], dt.bfloat16).ap()
    ob = nc.alloc_sbuf_tensor("ob", [TL, C * F], dt.bfloat16).ap()
    warm = nc.alloc_sbuf_tensor("warm", [1, 2], dt.float32).ap()

    ck = nc.alloc_psum_tensor("ck", [TL, KT], dt.float32).ap()
    tp0 = nc.alloc_psum_tensor("tp0", [128, TL], dt.bfloat16).ap()
    tp1 = nc.alloc_psum_tensor("tp1", [128, TL], dt.bfloat16).ap()
    po = [nc.alloc_psum_tensor(f"po{i}", [TL, 432], dt.float32).ap()
          for i in range(NPO)]

    identb = cmb[:, 0:128]
    maskb = cmb[:, 128:331]
    AF = mybir.ActivationFunctionType
    from contextlib import ExitStack

    with ExitStack() as stack:
        block = stack.enter_context(nc.Block(no_gpsimd_drain=True))
        names = ["cmsem", "sQa", "sQb", "sK1", "sK2", "sK3", "sK4", "sK5",
                 "sxA1", "sxA2", "sxB1", "sxB2", "tsem", "esem", "tpsem",
                 "asem", "rsem", "pub", "cqv", "cqs", "odsem", "wsem"]
        sem = {n: stack.enter_context(nc.semaphore(n)) for n in names}
        (cmsem, sQa, sQb, sK1, sK2, sK3, sK4, sK5, sxA1, sxA2, sxB1, sxB2,
         tsem, esem, tpsem, asem, rsem, pub, cqv, cqs, odsem, wsem) = (
            sem[n] for n in names)
        kga = dict(zip([lo for lo, _ in GA], [sK1, sK2, sK4]))
        kgb_s = dict(zip([lo for lo, _ in GB], [sK3, sK5]))
        kwait = {**kga, **kgb_s}

        @block.sync
        def _(sync):
            sync.dma_start(out=cmb[:], in_=cm_d[:]).then_inc(cmsem, 16)
            for (lo, hi), s in zip(GA, (sK1, sK2, sK4)):
                sync.dma_start(out=kgb[:, lo:hi, :],
                               in_=kg_d[:, lo:hi, :]).then_inc(s, 16)
            sync.dma_start(out=xr01[0:64, 0, :], in_=xr_d[0:64, :]).then_inc(sxA1, 16)
            sync.dma_start(out=xr01[0:38, 1, :], in_=xr_d[128:166, :]).then_inc(sxA2, 16)
            sync.wait_ge(cqv, 2)
            sync.wait_ge(cqs, 1)
            sync.dma_start(out=out_d[:, 0:OSPL],
                           in_=ob[:, 0:OSPL]).then_inc(odsem, 16)
            sync.wait_ge(odsem, 32)

        @block.scalar
        def _(scalar):
            # pre-load the exp + copy activation tables while DMA ramps
            scalar.wait_ge(wsem, 1)
            scalar.activation(warm[:, 0:1], warm[:, 0:1], AF.Exp)
            scalar.copy(warm[:, 1:2], warm[:, 1:2])
            scalar.dma_start(out=qb[:, 0:11, :], in_=q_d[:, 0:11, :]).then_inc(sQa, 16)
            scalar.dma_start(out=qb[:, 11:NCH, :], in_=q_d[:, 11:NCH, :]).then_inc(sQb, 16)
            for (lo, hi), s in zip(GB, (sK3, sK5)):
                scalar.dma_start(out=kgb[:, lo:hi, :],
                                 in_=kg_d[:, lo:hi, :]).then_inc(s, 16)
            scalar.dma_start(out=xr01[64:128, 0, :], in_=xr_d[64:128, :]).then_inc(sxB1, 16)
            scalar.dma_start(out=xr01[38:KT - 128, 1, :],
                             in_=xr_d[166:KT, :]).then_inc(sxB2, 16)
            # softmax exp straight off PSUM, split so transposes start early
            scalar.wait_ge(tsem, 1)
            scalar.activation(eb[:, 0:128], ck[:, 0:128], AF.Exp,
                              bias=0.0, scale=1.0).then_inc(esem, 1)
            scalar.activation(eb[:, 128:KT], ck[:, 128:KT], AF.Exp,
                              bias=0.0, scale=1.0).then_inc(esem, 1)
            # attention-weight transpose copy (lower part)
            scalar.wait_ge(tpsem, 1)
            scalar.copy(a1[:], tp1[0:KT - 128, :]).then_inc(asem, 1)
            # output copies: odd chunks; 1/rowsum folded into scale
            scalar.wait_ge(pub, 3)
            scalar.wait_ge(rsem, 2)
            scalar.activation(ob[:, VB[1]:VB[2]], po[1][:],
                              AF.Copy, bias=0.0, scale=rinv[:]).then_inc(cqs, 1)
            scalar.wait_ge(pub, 6)
            scalar.activation(ob[:, VB[3]:VB[4]], po[3][:],
                              AF.Copy, bias=0.0, scale=rinv[:]).then_inc(cqs, 1)
            scalar.wait_ge(pub, 6)
            scalar.activation(ob[:, VB[5]:VB[6]], po[0][:, 0:VB[6] - VB[5]],
                              AF.Copy, bias=0.0, scale=rinv[:]).then_inc(cqs, 1)
            # output piece 2 rides the scalar ring
            scalar.wait_ge(cqv, 3)
            scalar.wait_ge(cqs, 3)
            scalar.dma_start(out=out_d[:, OSPL:], in_=ob[:, OSPL:]).then_inc(odsem, 16)

        @block.tensor
        def _(tensor):
            # mask + leak corrections + conv bias enter the accumulation first
            tensor.wait_ge(cmsem, 16)
            tensor.matmul(ck[:], identb[0:TL, 0:TL], maskb[0:TL, :],
                          start=True, stop=False)
            tensor.wait_ge(sQa, 16)
            for cc in range(TOTCH):
                if cc in kwait:
                    tensor.wait_ge(kwait[cc], 16)
                if cc == 11:
                    tensor.wait_ge(sQb, 16)
                i, c = cc // NCH, cc % NCH
                tensor.matmul(ck[:], qb[:, c, i:i + TL], kgb[:, cc, :],
                              start=False, stop=(cc == TOTCH - 1))
            # drain fence publishes the finished score accumulation
            tensor.matmul(po[0][:, 0:128], kgb[:, 0, 0:TL], kgb[:, 0, 0:128],
                          start=True, stop=True).then_inc(tsem, 1)
            # transposes of attention weights + drain fence
            tensor.wait_ge(esem, 1)
            tensor.transpose(tp0[:], eb[:, 0:128], identb[0:TL, 0:TL])
            tensor.wait_ge(esem, 2)
            tensor.transpose(tp1[0:KT - 128, :], eb[:, 128:KT], identb[0:TL, 0:TL])
            tensor.matmul(ck[:, 0:128], identb[:, 0:TL], identb[:, 0:128],
                          start=True, stop=True).then_inc(tpsem, 1)
            # value matmuls: round A = chunks 0-2, round B = chunks 3-5;
            # two stationary loads per round (a0 then a1); chunk n is
            # published (drain-fenced) by the next >=128-col matmul
            tensor.wait_ge(asem, 2)
            tensor.wait_ge(sxA1, 16)
            tensor.wait_ge(sxB1, 16)
            for n in (0, 1, 2):
                tensor.matmul(po[n][:, 0:VB[n + 1] - VB[n]], a0[:, :],
                              xr01[:, 0, VB[n]:VB[n + 1]], start=True, stop=False)
            tensor.wait_ge(sxA2, 16)
            tensor.wait_ge(sxB2, 16)
            for n in (0, 1, 2):
                tensor.matmul(po[n][:, 0:VB[n + 1] - VB[n]], a1[:, :],
                              xr01[0:KT - 128, 1, VB[n]:VB[n + 1]],
                              start=False, stop=True)
            # round B's first matmul drain-fences round A; never attach
            # then_inc to a stop-matmul (its event does not fire post-drain)
            tensor.matmul(po[3][:, 0:VB[4] - VB[3]], a0[:, :],
                          xr01[:, 0, VB[3]:VB[4]],
                          start=True, stop=False).then_inc(pub, 3)  # chunks 0-2
            tensor.matmul(po[4][:, 0:VB[5] - VB[4]], a0[:, :],
                          xr01[:, 0, VB[4]:VB[5]], start=True, stop=False)
            tensor.wait_ge(cqv, 1)                # chunk 0's copy frees po[0]
            tensor.matmul(po[0][:, 0:VB[6] - VB[5]], a0[:, :],
                          xr01[:, 0, VB[5]:VB[6]], start=True, stop=False)
            for n in (3, 4, 5):
                tensor.matmul(po[n % NPO][:, 0:VB[n + 1] - VB[n]], a1[:, :],
                              xr01[0:KT - 128, 1, VB[n]:VB[n + 1]],
                              start=False, stop=True)
            tensor.matmul(ck[:, 0:128], identb[:, 0:TL], identb[:, 0:128],
                          start=True, stop=True).then_inc(pub, 3)   # chunks 3-5

        @block.vector
        def _(vector):
            vector.memset(warm[:], 0.0).then_inc(wsem, 1)
            # attention-weight transpose copy (upper part)
            vector.wait_ge(tpsem, 1)
            vector.tensor_copy(a0[:], tp0[:]).then_inc(asem, 1)
            # row sums + reciprocal (off the transpose critical path)
            vector.tensor_reduce(ssum[:], eb[:], axis=mybir.AxisListType.X,
                                 op=mybir.AluOpType.add).then_inc(rsem, 1)
            vector.wait_ge(rsem, 1)
            vector.reciprocal(rinv[:], ssum[:]).then_inc(rsem, 1)
            # output copies: even chunks
            vector.wait_ge(rsem, 2)
            vector.wait_ge(pub, 3)
            vector.tensor_scalar_mul(ob[:, VB[0]:VB[1]], po[0][:],
                                     rinv[:]).then_inc(cqv, 1)
            vector.tensor_scalar_mul(ob[:, VB[2]:VB[3]], po[2][:],
                                     rinv[:]).then_inc(cqv, 1)
            vector.wait_ge(pub, 6)
            vector.tensor_scalar_mul(ob[:, VB[4]:VB[5]], po[4][:],
                                     rinv[:]).then_inc(cqv, 1)

    nc.compile()
    _CACHE["ncr"] = nc
    return nc


def _host_prep(x_mic, x_ref, w_mic, b_mic, w_ref, b_ref, w_conv, b_conv):
    """Build the 8 per-core input maps (layout prep + tiny 1x1 projections)."""
    f32 = np.float32
    wc = w_conv[0]                                   # (H, 5, 3)
    Qh = np.einsum("hc,bctf->bhtf", w_mic, x_mic) + b_mic[None, :, None, None]
    Kh = np.einsum("hc,bctf->bhtf", w_ref, x_ref) + b_ref[None, :, None, None]
    PAD = 120
    Khp = np.pad(Kh, ((0, 0), (0, 0), (PAD, PAD), (0, 0)))
    Qhp = np.pad(Qh, ((0, 0), (0, 0), (8, 8), (0, 0)))
    xrp = np.pad(x_ref, ((0, 0), (0, 0), (PAD, PAD), (0, 0)))
    L = T + 2 * PAD
    # KGg[i][b,h,m,f] = sum_j' wc[h,i,j'] Khp[m + j'], tau(m) = m + 1 - PAD
    KGg = np.zeros((NV, B, H, L - 2, F), f32)
    for i in range(NV):
        for jp in range(3):
            KGg[i] += wc[:, i, jp][None, :, None, None] * Khp[:, :, jp:jp + L - 2, :]

    cm = np.zeros((128, 331), f32)
    cm[:, 0:128] = np.eye(128, dtype=f32)
    in_maps, core_meta = [], []
    for b in range(B):
        for tc in range(T // TL):
            t0 = tc * TL
            Qb = Qhp[b][:, t0 + 4:t0 + 4 + QT, :]            # x' in [-4, 100)
            qrows = Qb.transpose(0, 2, 1).reshape(H * F, QT) / KSCALE
            qp = np.zeros((NCH * 128, QT), f32)
            qp[:H * F] = qrows
            qpack = np.ascontiguousarray(
                qp.reshape(NCH, 128, QT).transpose(1, 0, 2)).astype(BF16)
            # K variants, column-shifted so all matmuls read cols [0, KT)
            kgp = np.zeros((TOTCH, 128, KT), f32)
            for i in range(NV):
                m0 = t0 - 108 + i + PAD                      # tau = t0-107+i+j2
                sl = KGg[i, b][:, m0:m0 + KT, :]
                rows = sl.transpose(0, 2, 1).reshape(H * F, KT) * KSCALE
                tmp = np.zeros((NCH * 128, KT), f32)
                tmp[:H * F] = rows
                kgp[i * NCH:(i + 1) * NCH] = tmp.reshape(NCH, 128, KT)
            kgpack = np.ascontiguousarray(kgp.transpose(1, 0, 2)).astype(FP8)
            # additive mask: band + exact d-edge leak corrections + conv bias
            x_idx = np.arange(TL)[:, None]
            j_idx = np.arange(KT)[None, :]
            band = (j_idx >= x_idx + 4) & (j_idx <= x_idx + 103)
            mask = np.where(band, 0.0, NEG).astype(f32)
            xs = np.arange(-4, TL)
            Dm1 = np.einsum("hxf,hxf->hx", Qb, Khp[b][:, t0 + xs - 100 + PAD, :])
            Dp1 = np.einsum("hxf,hxf->hx", Qb, Khp[b][:, t0 + xs + 1 + PAD, :])
            xv = np.arange(TL)
            leak0 = np.zeros(TL, f32)
            leak99 = np.zeros(TL, f32)
            for i in range(NV):
                leak0 += wc[:, i, 0] @ Dm1[:, xv + i]
                leak99 += wc[:, i, 2] @ Dp1[:, xv + i]
            mask[xv, xv + 4] -= leak0
            mask[xv, xv + 103] -= leak99
            mask += float(np.asarray(b_conv).reshape(-1)[0])
            cmc = cm.copy()
            cmc[:TL, 128:331] = mask
            # raw x_ref windows for the value matmul: [j, (c, f)]
            jt = t0 - 103 + np.arange(KT)
            xrb = np.ascontiguousarray(
                xrp[b][:, jt + PAD, :].transpose(1, 0, 2).reshape(KT, C * F)
            ).astype(BF16)
            in_maps.append({
                "cm": cmc.astype(BF16), "qf": qpack, "kg": kgpack, "xr": xrb,
            })
            core_meta.append((b, t0))
    return in_maps, core_meta


def kernel(**inputs):
    x_mic = np.asarray(inputs["x_mic"], dtype=np.float32)
    x_ref = np.asarray(inputs["x_ref"], dtype=np.float32)
    w_mic = np.asarray(inputs["w_mic"], dtype=np.float32)
    b_mic = np.asarray(inputs["b_mic"], dtype=np.float32)
    w_ref = np.asarray(inputs["w_ref"], dtype=np.float32)
    b_ref = np.asarray(inputs["b_ref"], dtype=np.float32)
    w_conv = np.asarray(inputs["w_conv"], dtype=np.float32)
    b_conv = np.asarray(inputs["b_conv"], dtype=np.float32)
    delay = int(inputs["delay"])
    assert delay == DELAY, f"kernel hardcodes delay={DELAY}, got {delay}"

    in_maps, core_meta = _host_prep(
        x_mic, x_ref, w_mic, b_mic, w_ref, b_ref, w_conv, b_conv
    )
    nc = _build_raw()
    from concourse.bass_utils import run_bass_kernel_spmd

    res = run_bass_kernel_spmd(nc, in_maps, core_ids=list(range(8)))
    out = np.zeros((B, C, T, F), dtype=np.float32)
    for (b, t0), r in zip(core_meta, res.results):
        o = np.asarray(r["out"], dtype=np.float32).reshape(TL, C, F)
        out[b, :, t0:t0 + TL, :] = o.transpose(1, 0, 2)
    return out


if __name__ == "__main__":
    z = np.load("/tmp/inputs.npz")
    ins = {k: z[k] for k in z.files}
    out = kernel(**ins)
    ref = np.load("/tmp/ref.npy")
    rel = np.abs(out - ref).max() / np.abs(ref).max()
    print("Relative error:", rel)


# revision 18
# speedup vs baseline: 1.3628x; 1.3628x over previous
"""AlignBlock Trainium2 kernel — 8-core SPMD, no collectives.

Sharding: 8 cores = 2 batch x 4 time-chunks of 100 steps, fully independent
(halo-included input slices).

Device algorithm per core ("shifted K-conv variants"):
  The 5x3 conv over (t, d) of the QK^T scores is folded EXACTLY into the
  score matmul by pre-convolving the K projection with the 3 d-taps for each
  of the 5 time taps i:

      KG_i[k, y] = sum_j' wc[h,i,j'] * Kh[k, y + j' - 1]          (k = (h,f))
      Ck[x, j]   = sum_i sum_k Q[k, x+i-4] * KG_i[k, j+i-4]

  The Q-side time shifts are free SBUF column offsets of one shared Q buffer
  (104 cols); the K-side shifts are baked into each variant's column layout.
  This ships 5x fp8 K-variants + 1x bf16 Q (3.4 MB) instead of rank-5 SVD
  factors of both sides (4.8 MB). KG is scaled by 64 (inverse folded into
  bf16 Q) to clear fp8e4m3's subnormal floor.

  The additive softmax mask (band + exact d-edge leak corrections + conv
  bias) is folded into the same PSUM accumulation as an identity-weighted
  bf16 matmul, so softmax is just exp() on ACT straight out of PSUM (logits
  bounded, no max pass). Attention weights are transposed on the PE and
  applied to raw bf16 x_ref windows in two stationary-weight rounds over 6
  output column chunks; 1/rowsum rides the PSUM->SBUF output copies (bf16
  out, halving the store).

  DMA notes: all bulk transfers use >=2.6KB-per-partition descriptors
  (smaller descriptors measurably drop per-engine DMA throughput); inputs
  are split across both HWDGE rings roughly proportional to when the
  tensor engine consumes them; the two output pieces ride opposite rings.
"""

import numpy as np
import ml_dtypes

B, C, H, T, F, DELAY = 2, 16, 16, 400, 161, 100
TL = 100            # output timesteps per core
QT = TL + 4         # mic-side cols (causal conv halo)
KT = TL + 103       # ref-side cols (window + conv halos)
NV = 5              # conv time taps = K variants
NCH = 21            # 128-row chunks per variant (H*F = 2576 rows)
TOTCH = NV * NCH    # 105
KSCALE = 64.0       # fp8 pre-scale on KG, inverse folded into Q
NEG = -60.0         # out-of-band additive mask
VB = [0, 432, 864, 1296, 1728, 2160, 2576]   # value/output column chunks
NPO = 5             # PSUM banks for value chunks (chunk 5 reuses bank 0)
GA = [(0, 20), (46, 82)]               # KG chunk groups on the sync ring
GB = [(20, 46), (82, 105)]             # KG chunk groups on the scalar ring
OSPL = VB[3]        # output piece split (chunks 0-2 / 3-5)

BF16 = ml_dtypes.bfloat16
FP8 = ml_dtypes.float8_e4m3

_CACHE = {}


def _build_raw():
    if "ncr" in _CACHE:
        return _CACHE["ncr"]
    import concourse.bass as bass
    from concourse import bacc, mybir

    dt = mybir.dt
    nc = bacc.Bacc("TRN2", target_bir_lowering=False, debug=False, num_devices=8)

    cm_d = nc.dram_tensor("cm", [128, 331], dt.bfloat16, kind="ExternalInput").ap()
    q_d = nc.dram_tensor("qf", [128, NCH, QT], dt.bfloat16, kind="ExternalInput").ap()
    kg_d = nc.dram_tensor("kg", [128, TOTCH, KT], dt.float8e4, kind="ExternalInput").ap()
    xr_d = nc.dram_tensor("xr", [KT, C * F], dt.bfloat16, kind="ExternalInput").ap()
    out_d = nc.dram_tensor("out", [TL, C * F], dt.bfloat16, kind="ExternalOutput").ap()

    # static SBUF
    cmb = nc.alloc_sbuf_tensor("cmb", [128, 331], dt.bfloat16).ap()
    qb = nc.alloc_sbuf_tensor("qb", [128, NCH, QT], dt.bfloat16).ap()
    kgb = nc.alloc_sbuf_tensor("kgb", [128, TOTCH, KT], dt.float8e4).ap()
    xr01 = nc.alloc_sbuf_tensor("xr01", [128, 2, C * F], dt.bfloat16).ap()
    eb = nc.alloc_sbuf_tensor("eb", [TL, KT], dt.bfloat16).ap()
    ssum = nc.alloc_sbuf_tensor("ssum", [TL, 1], dt.float32).ap()
    rinv = nc.alloc_sbuf_tensor("rinv", [TL, 1], dt.float32).ap()
    a0 = nc.alloc_sbuf_tensor("a0", [128, TL], dt.bfloat16).ap()
    a1 = nc.alloc_sbuf_tensor("a1", [KT - 128, TL], dt.bfloat16).ap()
    ob = nc.alloc_sbuf_tensor("ob", [TL, C * F], dt.bfloat16).ap()
    warm = nc.alloc_sbuf_tensor("warm", [1, 2], dt.float32).ap()

    ck = nc.alloc_psum_tensor("ck", [TL, KT], dt.float32).ap()
    tp0 = nc.alloc_psum_tensor("tp0", [128, TL], dt.bfloat16).ap()
    tp1 = nc.alloc_psum_tensor("tp1", [128, TL], dt.bfloat16).ap()
    po = [nc.alloc_psum_tensor(f"po{i}", [TL, 432], dt.float32).ap()
          for i in range(NPO)]

    identb = cmb[:, 0:128]
    maskb = cmb[:, 128:331]
    AF = mybir.ActivationFunctionType
    from contextlib import ExitStack

    with ExitStack() as stack:
        block = stack.enter_context(nc.Block(no_gpsimd_drain=True))
        names = ["cmsem", "sQ", "sK1", "sK2", "sK3", "sK4",
                 "sxA", "sxB", "tsem", "esem", "tpsem",
                 "asem", "rsem", "pub", "cqv", "cqs", "odsem", "wsem"]
        sem = {n: stack.enter_context(nc.semaphore(n)) for n in names}
        (cmsem, sQ, sK1, sK2, sK3, sK4, sxA, sxB,
         tsem, esem, tpsem, asem, rsem, pub, cqv, cqs, odsem, wsem) = (
            sem[n] for n in names)
        kwait = {GA[0][0]: sK1, GB[0][0]: sK2, GA[1][0]: sK3, GB[1][0]: sK4}

        @block.sync
        def _(sync):
            sync.dma_start(out=cmb[:], in_=cm_d[:]).then_inc(cmsem, 16)
            for (lo, hi), s in zip(GA, (sK1, sK3)):
                sync.dma_start(out=kgb[:, lo:hi, :],
                               in_=kg_d[:, lo:hi, :]).then_inc(s, 16)
            sync.dma_start(out=xr01[:, 0, :], in_=xr_d[0:128, :]).then_inc(sxA, 16)
            sync.wait_ge(cqv, 2)
            sync.wait_ge(cqs, 1)
            sync.dma_start(out=out_d[:, 0:OSPL],
                           in_=ob[:, 0:OSPL]).then_inc(odsem, 16)
            sync.wait_ge(odsem, 32)

        @block.scalar
        def _(scalar):
            # pre-load the exp + copy activation tables while DMA ramps
            scalar.wait_ge(wsem, 1)
            scalar.activation(warm[:, 0:1], warm[:, 0:1], AF.Exp)
            scalar.copy(warm[:, 1:2], warm[:, 1:2])
            scalar.dma_start(out=qb[:], in_=q_d[:]).then_inc(sQ, 16)
            for (lo, hi), s in zip(GB, (sK2, sK4)):
                scalar.dma_start(out=kgb[:, lo:hi, :],
                                 in_=kg_d[:, lo:hi, :]).then_inc(s, 16)
            scalar.dma_start(out=xr01[0:KT - 128, 1, :],
                             in_=xr_d[128:KT, :]).then_inc(sxB, 16)
            # softmax exp straight off PSUM, split so transposes start early
            scalar.wait_ge(tsem, 1)
            scalar.activation(eb[:, 0:128], ck[:, 0:128], AF.Exp,
                              bias=0.0, scale=1.0).then_inc(esem, 1)
            scalar.activation(eb[:, 128:KT], ck[:, 128:KT], AF.Exp,
                              bias=0.0, scale=1.0).then_inc(esem, 1)
            # attention-weight transpose copy (lower part)
            scalar.wait_ge(tpsem, 1)
            scalar.copy(a1[:], tp1[0:KT - 128, :]).then_inc(asem, 1)
            # output copies: odd chunks; 1/rowsum folded into scale
            scalar.wait_ge(pub, 3)
            scalar.wait_ge(rsem, 2)
            scalar.activation(ob[:, VB[1]:VB[2]], po[1][:],
                              AF.Copy, bias=0.0, scale=rinv[:]).then_inc(cqs, 1)
            scalar.wait_ge(pub, 6)
            scalar.activation(ob[:, VB[3]:VB[4]], po[3][:],
                              AF.Copy, bias=0.0, scale=rinv[:]).then_inc(cqs, 1)
            scalar.wait_ge(pub, 6)
            scalar.activation(ob[:, VB[5]:VB[6]], po[0][:, 0:VB[6] - VB[5]],
                              AF.Copy, bias=0.0, scale=rinv[:]).then_inc(cqs, 1)
            # output piece 2 rides the scalar ring
            scalar.wait_ge(cqv, 3)
            scalar.wait_ge(cqs, 3)
            scalar.dma_start(out=out_d[:, OSPL:], in_=ob[:, OSPL:]).then_inc(odsem, 16)

        @block.tensor
        def _(tensor):
            # mask + leak corrections + conv bias enter the accumulation first
            tensor.wait_ge(cmsem, 16)
            tensor.matmul(ck[:], identb[0:TL, 0:TL], maskb[0:TL, :],
                          start=True, stop=False)
            tensor.wait_ge(sQ, 16)
            for cc in range(TOTCH):
                if cc in kwait:
                    tensor.wait_ge(kwait[cc], 16)
                i, c = cc // NCH, cc % NCH
                tensor.matmul(ck[:], qb[:, c, i:i + TL], kgb[:, cc, :],
                              start=False, stop=(cc == TOTCH - 1))
            # drain fence publishes the finished score accumulation
            tensor.matmul(po[0][:, 0:128], kgb[:, 0, 0:TL], kgb[:, 0, 0:128],
                          start=True, stop=True).then_inc(tsem, 1)
            # transposes of attention weights + drain fence
            tensor.wait_ge(esem, 1)
            tensor.transpose(tp0[:], eb[:, 0:128], identb[0:TL, 0:TL])
            tensor.wait_ge(esem, 2)
            tensor.transpose(tp1[0:KT - 128, :], eb[:, 128:KT], identb[0:TL, 0:TL])
            tensor.matmul(ck[:, 0:128], identb[:, 0:TL], identb[:, 0:128],
                          start=True, stop=True).then_inc(tpsem, 1)
            # value matmuls: round A = chunks 0-2, round B = chunks 3-5;
            # two stationary loads per round (a0 then a1); chunk n is
            # published (drain-fenced) by the next >=128-col matmul
            tensor.wait_ge(asem, 2)
            tensor.wait_ge(sxA, 16)
            for n in (0, 1, 2):
                tensor.matmul(po[n][:, 0:VB[n + 1] - VB[n]], a0[:, :],
                              xr01[:, 0, VB[n]:VB[n + 1]], start=True, stop=False)
            tensor.wait_ge(sxB, 16)
            for n in (0, 1, 2):
                tensor.matmul(po[n][:, 0:VB[n + 1] - VB[n]], a1[:, :],
                              xr01[0:KT - 128, 1, VB[n]:VB[n + 1]],
                              start=False, stop=True)
            # round B's first matmul drain-fences round A; never attach
            # then_inc to a stop-matmul (its event does not fire post-drain)
            tensor.matmul(po[3][:, 0:VB[4] - VB[3]], a0[:, :],
                          xr01[:, 0, VB[3]:VB[4]],
                          start=True, stop=False).then_inc(pub, 3)  # chunks 0-2
            tensor.matmul(po[4][:, 0:VB[5] - VB[4]], a0[:, :],
                          xr01[:, 0, VB[4]:VB[5]], start=True, stop=False)
            tensor.wait_ge(cqv, 1)                # chunk 0's copy frees po[0]
            tensor.matmul(po[0][:, 0:VB[6] - VB[5]], a0[:, :],
                          xr01[:, 0, VB[5]:VB[6]], start=True, stop=False)
            for n in (3, 4, 5):
                tensor.matmul(po[n % NPO][:, 0:VB[n + 1] - VB[n]], a1[:, :],
                              xr01[0:KT - 128, 1, VB[n]:VB[n + 1]],
                              start=False, stop=True)
            tensor.matmul(ck[:, 0:128], identb[:, 0:TL], identb[:, 0:128],
                          start=True, stop=True).then_inc(pub, 3)   # chunks 3-5

        @block.vector
        def _(vector):
            vector.memset(warm[:], 0.0).then_inc(wsem, 1)
            # attention-weight transpose copy (upper part)
            vector.wait_ge(tpsem, 1)
            vector.tensor_copy(a0[:], tp0[:]).then_inc(asem, 1)
            # row sums + reciprocal (off the transpose critical path)
            vector.tensor_reduce(ssum[:], eb[:], axis=mybir.AxisListType.X,
                                 op=mybir.AluOpType.add).then_inc(rsem, 1)
            vector.wait_ge(rsem, 1)
            vector.reciprocal(rinv[:], ssum[:]).then_inc(rsem, 1)
            # output copies: even chunks
            vector.wait_ge(rsem, 2)
            vector.wait_ge(pub, 3)
            vector.tensor_scalar_mul(ob[:, VB[0]:VB[1]], po[0][:],
                                     rinv[:]).then_inc(cqv, 1)
            vector.tensor_scalar_mul(ob[:, VB[2]:VB[3]], po[2][:],
                                     rinv[:]).then_inc(cqv, 1)
            vector.wait_ge(pub, 6)
            vector.tensor_scalar_mul(ob[:, VB[4]:VB[5]], po[4][:],
                                     rinv[:]).then_inc(cqv, 1)

    nc.compile()
    _CACHE["ncr"] = nc
    return nc


def _host_prep(x_mic, x_ref, w_mic, b_mic, w_ref, b_ref, w_conv, b_conv):
    """Build the 8 per-core input maps (layout prep + tiny 1x1 projections)."""
    f32 = np.float32
    wc = w_conv[0]                                   # (H, 5, 3)
    Qh = np.einsum("hc,bctf->bhtf", w_mic, x_mic) + b_mic[None, :, None, None]
    Kh = np.einsum("hc,bctf->bhtf", w_ref, x_ref) + b_ref[None, :, None, None]
    PAD = 120
    Khp = np.pad(Kh, ((0, 0), (0, 0), (PAD, PAD), (0, 0)))
    Qhp = np.pad(Qh, ((0, 0), (0, 0), (8, 8), (0, 0)))
    xrp = np.pad(x_ref, ((0, 0), (0, 0), (PAD, PAD), (0, 0)))
    L = T + 2 * PAD
    # KGg[i][b,h,m,f] = sum_j' wc[h,i,j'] Khp[m + j'], tau(m) = m + 1 - PAD
    KGg = np.zeros((NV, B, H, L - 2, F), f32)
    for i in range(NV):
        for jp in range(3):
            KGg[i] += wc[:, i, jp][None, :, None, None] * Khp[:, :, jp:jp + L - 2, :]

    cm = np.zeros((128, 331), f32)
    cm[:, 0:128] = np.eye(128, dtype=f32)
    in_maps, core_meta = [], []
    for b in range(B):
        for tc in range(T // TL):
            t0 = tc * TL
            Qb = Qhp[b][:, t0 + 4:t0 + 4 + QT, :]            # x' in [-4, 100)
            qrows = Qb.transpose(0, 2, 1).reshape(H * F, QT) / KSCALE
            qp = np.zeros((NCH * 128, QT), f32)
            qp[:H * F] = qrows
            qpack = np.ascontiguousarray(
                qp.reshape(NCH, 128, QT).transpose(1, 0, 2)).astype(BF16)
            # K variants, column-shifted so all matmuls read cols [0, KT)
            kgp = np.zeros((TOTCH, 128, KT), f32)
            for i in range(NV):
                m0 = t0 - 108 + i + PAD                      # tau = t0-107+i+j2
                sl = KGg[i, b][:, m0:m0 + KT, :]
                rows = sl.transpose(0, 2, 1).reshape(H * F, KT) * KSCALE
                tmp = np.zeros((NCH * 128, KT), f32)
                tmp[:H * F] = rows
                kgp[i * NCH:(i + 1) * NCH] = tmp.reshape(NCH, 128, KT)
            kgpack = np.ascontiguousarray(kgp.transpose(1, 0, 2)).astype(FP8)
            # additive mask: band + exact d-edge leak corrections + conv bias
            x_idx = np.arange(TL)[:, None]
            j_idx = np.arange(KT)[None, :]
            band = (j_idx >= x_idx + 4) & (j_idx <= x_idx + 103)
            mask = np.where(band, 0.0, NEG).astype(f32)
            xs = np.arange(-4, TL)
            Dm1 = np.einsum("hxf,hxf->hx", Qb, Khp[b][:, t0 + xs - 100 + PAD, :])
            Dp1 = np.einsum("hxf,hxf->hx", Qb, Khp[b][:, t0 + xs + 1 + PAD, :])
            xv = np.arange(TL)
            leak0 = np.zeros(TL, f32)
            leak99 = np.zeros(TL, f32)
            for i in range(NV):
                leak0 += wc[:, i, 0] @ Dm1[:, xv + i]
                leak99 += wc[:, i, 2] @ Dp1[:, xv + i]
            mask[xv, xv + 4] -= leak0
            mask[xv, xv + 103] -= leak99
            mask += float(np.asarray(b_conv).reshape(-1)[0])
            cmc = cm.copy()
            cmc[:TL, 128:331] = mask
            # raw x_ref windows for the value matmul: [j, (c, f)]
            jt = t0 - 103 + np.arange(KT)
            xrb = np.ascontiguousarray(
                xrp[b][:, jt + PAD, :].transpose(1, 0, 2).reshape(KT, C * F)
            ).astype(BF16)
            in_maps.append({
                "cm": cmc.astype(BF16), "qf": qpack, "kg": kgpack, "xr": xrb,
            })
            core_meta.append((b, t0))
    return in_maps, core_meta


def kernel(**inputs):
    x_mic = np.asarray(inputs["x_mic"], dtype=np.float32)
    x_ref = np.asarray(inputs["x_ref"], dtype=np.float32)
    w_mic = np.asarray(inputs["w_mic"], dtype=np.float32)
    b_mic = np.asarray(inputs["b_mic"], dtype=np.float32)
    w_ref = np.asarray(inputs["w_ref"], dtype=np.float32)
    b_ref = np.asarray(inputs["b_ref"], dtype=np.float32)
    w_conv = np.asarray(inputs["w_conv"], dtype=np.float32)
    b_conv = np.asarray(inputs["b_conv"], dtype=np.float32)
    delay = int(inputs["delay"])
    assert delay == DELAY, f"kernel hardcodes delay={DELAY}, got {delay}"

    in_maps, core_meta = _host_prep(
        x_mic, x_ref, w_mic, b_mic, w_ref, b_ref, w_conv, b_conv
    )
    nc = _build_raw()
    from concourse.bass_utils import run_bass_kernel_spmd

    res = run_bass_kernel_spmd(nc, in_maps, core_ids=list(range(8)))
    out = np.zeros((B, C, T, F), dtype=np.float32)
    for (b, t0), r in zip(core_meta, res.results):
        o = np.asarray(r["out"], dtype=np.float32).reshape(TL, C, F)
        out[b, :, t0:t0 + TL, :] = o.transpose(1, 0, 2)
    return out


if __name__ == "__main__":
    z = np.load("/tmp/inputs.npz")
    ins = {k: z[k] for k in z.files}
    out = kernel(**ins)
    ref = np.load("/tmp/ref.npy")
    rel = np.abs(out - ref).max() / np.abs(ref).max()
    print("Relative error:", rel)
